# revision 68
# baseline (speedup 1.0000x reference)
"""Multi-head attention (B=2, SQ=SK=2048, D=1024, H=16, DK=64) on 8 TRN2 cores.

Sharding: core c handles batch b = c//4 and head-group hg = c%4 (4 heads,
256 feature columns of each projection).  Each core computes its heads'
Q/K/V projections, causal+padding-masked softmax attention, and a partial
output projection; the host sums the 4 partials per batch.

All matmul operands are bf16 (1 cycle/row on the PE).  Device layouts:
  qT/kT  [128, m, tok]  packed: feature block m holds heads 2m (partitions
                        0-63) and 2m+1 (64-127) -- exactly the projection
                        psum layout, so evictions are plain copies.
  v      [tok, dk+1]    natural per head, padding mask folded into the rows;
                        the extra "masked ones" column makes the AV matmul
                        emit the softmax denominator for free.
  sT     [ktok, qtok]   transposed scores in PSUM; the two heads of a pair
                        run as K=64 row-tiled matmuls (partitions 0-63 /
                        64-127) that execute concurrently in the PE array.
  ctxT   [65, qtok]     accumulated over ktok tiles (row 64 = denominator).

Causality is exploited at 128-token granularity: score/AV/exp work for a
k-tile only covers valid queries (free dim trimmed), and the diagonal
128x128 triangle is zeroed via affine_select after exp.  Softmax runs
without max subtraction (scores are O(6) for randn inputs).  The Q
projection is emitted per 512-token chunk, interleaved with attention, so
the scalar engine's exp stream starts early; the output projection of
chunk qc-1 is interleaved into chunk qc's attention to fill PE gaps.
"""

import numpy as np

B, SQ, SK, D, H, DK = 2, 2048, 2048, 1024, 16, 64
N_CORES = 8
CORES_PER_BATCH = 4
DKC = D // CORES_PER_BATCH          # 256 projection columns per core
QCH = 512                           # q-chunk (moving free dim)
ONES_EPS = 1e-20

_PROG_CACHE = {}


def _build(cfg):
    """Build the per-core Bass program. cfg = (sq, sk, d, dkc)."""
    import concourse.bass as bass  # noqa: F401
    import concourse.mybir as mybir
    import concourse.tile as tile
    from concourse import bacc
    from contextlib import ExitStack

    f32 = mybir.dt.float32
    bf16 = mybir.dt.bfloat16
    i32 = mybir.dt.int32
    Exp = mybir.ActivationFunctionType.Exp
    mult = mybir.AluOpType.mult
    is_ge = mybir.AluOpType.is_ge

    sq, sk, d, dkc = cfg
    kc_n = d // 128                  # contraction chunks for projections
    mc_n = dkc // 128                # head pairs (128-feature blocks)
    kt_n = sk // 128                 # key tiles
    qc_n = sq // QCH                 # q chunks
    hpc = dkc // DK                  # heads per core
    vw = DK + 1                      # v row width per head incl. ones col
    fc_n = d // 512                  # output feature chunks

    nc = bacc.Bacc("TRN2", target_bir_lowering=False, debug=False,
                   enable_asserts=False, num_devices=N_CORES)

    xqT = nc.dram_tensor("xqT", [d, sq], bf16, kind="ExternalInput").ap()
    xkT = nc.dram_tensor("xkT", [d, sk], bf16, kind="ExternalInput").ap()
    xvT = nc.dram_tensor("xvT", [d, sk], bf16, kind="ExternalInput").ap()
    wq_d = nc.dram_tensor("wq", [d, dkc], bf16, kind="ExternalInput").ap()
    wk_d = nc.dram_tensor("wk", [d, dkc], bf16, kind="ExternalInput").ap()
    wv_d = nc.dram_tensor("wv", [d, dkc], bf16, kind="ExternalInput").ap()
    wo_d = nc.dram_tensor("wo", [dkc, d], bf16, kind="ExternalInput").ap()
    mask_d = nc.dram_tensor("maskb", [sk], i32, kind="ExternalInput").ap()
    out_d = nc.dram_tensor("out", [sq, d], bf16, kind="ExternalOutput").ap()

    with tile.TileContext(nc) as tc, ExitStack() as ctx:
        const = ctx.enter_context(tc.tile_pool(name="const", bufs=1))
        wpool = ctx.enter_context(tc.tile_pool(name="wpool", bufs=4))
        xpool = ctx.enter_context(tc.tile_pool(name="xpool", bufs=1))
        ptp = ctx.enter_context(tc.tile_pool(name="ptp", bufs=4))
        outp = ctx.enter_context(tc.tile_pool(name="outp", bufs=2))
        nrm = ctx.enter_context(tc.tile_pool(name="nrm", bufs=2))
        cbp = ctx.enter_context(tc.tile_pool(name="cbp", bufs=2))
        sbp = ctx.enter_context(tc.tile_pool(name="sbp", bufs=2,
                                             space="PSUM"))
        prjp = ctx.enter_context(tc.tile_pool(name="prjp", bufs=2,
                                              space="PSUM"))
        ctp = ctx.enter_context(tc.tile_pool(name="ctp", bufs=2,
                                             space="PSUM"))

        # ---------------- DMA everything up-front.  xv is issued from the
        # (otherwise idle) vector engine at fine grain so the V projection
        # starts early; xk/xq go on sync after the weights.  Each dma_start
        # lands on one HW queue (~22 GB/s), so tensors are split into many
        # sub-DMAs that spread across the 16 queues.
        mask_i = const.tile([128, kt_n], i32, tag="mask_i")
        nc.sync.dma_start(mask_i[:], mask_d.rearrange("(t p) -> p t", p=128))

        wv_sb = wpool.tile([128, kc_n, dkc], bf16, tag="w")
        wv_r = wv_d.rearrange("(c p) m -> p c m", p=128)
        wk_sb = wpool.tile([128, kc_n, dkc], bf16, tag="w")
        wk_r = wk_d.rearrange("(c p) m -> p c m", p=128)
        wq_sb = wpool.tile([128, kc_n, dkc], bf16, tag="w")
        wq_r = wq_d.rearrange("(c p) m -> p c m", p=128)
        wo_sb = wpool.tile([128, mc_n, fc_n, 512], bf16, tag="w")
        wo_r = wo_d.rearrange("(c p) (f n) -> p c f n", p=128, n=512)

        def w_issues(dst, src_r, n):
            return [(lambda c=c: nc.sync.dma_start(dst[:, c], src_r[:, c]))
                    for c in range(n)]

        _xt = [0]

        def alloc_x(ntok):
            """Per-(c, segment) tiles: 512-token segments for the first two
            chunks (fast arrival), 1024 afterwards.  Returns entries at
            512-token granularity plus the tile list grouped by segment."""
            segs = []
            t = 0
            while t < ntok:
                w = 512 if (t < 1024 or ntok - t < 1024) else 1024
                segs.append((t, w))
                t += w
            xs = [[None] * (ntok // 512) for _ in range(kc_n)]
            groups = []
            for t0, w in segs:
                grp = []
                for c in range(kc_n):
                    _xt[0] += 1
                    tl = xpool.tile([128, w], bf16, tag=f"x{_xt[0]}",
                                    name="xc")
                    grp.append((tl, c, t0, w))
                    for s in range(w // 512):
                        xs[c][t0 // 512 + s] = (tl, s * 512)
                groups.append(grp)
            return xs, groups

        # All input DMAs issue on sync, interleaved segment-major across
        # the three X tensors in first-consumption order.
        xv, xv_g = alloc_x(sk)
        xk, xk_g = alloc_x(sk)
        xq, xq_g = alloc_x(sq)

        def seg(groups, i):
            return groups[i] if i < len(groups) else []

        issue_order = []
        for dram, grp in [(None, w_issues(wv_sb, wv_r, kc_n)),
                          (xvT, seg(xv_g, 0)),
                          (None, w_issues(wk_sb, wk_r, kc_n)),
                          (xkT, seg(xk_g, 0)),
                          (None, w_issues(wq_sb, wq_r, kc_n)),
                          (xqT, seg(xq_g, 0)),
                          (xvT, seg(xv_g, 1)),
                          (xkT, seg(xk_g, 1)), (xqT, seg(xq_g, 1)),
                          (xvT, seg(xv_g, 2)), (xkT, seg(xk_g, 2)),
                          (None, w_issues(wo_sb, wo_r, mc_n)),
                          (xvT, seg(xv_g, 3)), (xqT, seg(xq_g, 2))]:
            issue_order += [(dram, a) for a in grp]
        seen = {id(a) for _, a in issue_order}
        for groups, dram in ((xv_g, xvT), (xk_g, xkT), (xq_g, xqT)):
            for grp in groups:
                issue_order += [(dram, a) for a in grp if id(a) not in seen]
        for dram, a in issue_order:
            if dram is None:
                a()
            else:
                tl, c, t0, w = a
                nc.sync.dma_start(tl[:],
                                  dram[c * 128:(c + 1) * 128, t0:t0 + w])

        # ---------------- constants / persistent tensors
        mask01 = const.tile([128, kt_n], f32, tag="mask01")
        nc.vector.tensor_copy(mask01[:], mask_i[:])
        mask01p = const.tile([128, kt_n], f32, tag="mask01p")
        nc.vector.tensor_scalar_add(mask01p[:], mask01[:], ONES_EPS)
        # causal-bias constants: negid = -800*I, ltri = strict lower
        # triangle (ones below the diagonal).  The diagonal score tiles get
        # `negid.T @ ltri` (= -800 where q_local < k_local) accumulated into
        # PSUM before exp, so exp underflows the future positions to zero
        # without any gpsimd work on the critical path.
        ones128 = const.tile([128, 128], f32, tag="ones128")
        nc.vector.memset(ones128[:], 1.0)
        geq0 = const.tile([128, 128], f32, tag="geq0")
        nc.gpsimd.affine_select(out=geq0[:], in_=ones128[:],
                                compare_op=is_ge, fill=0.0,
                                base=0, channel_multiplier=-1,
                                pattern=[[1, 128]])
        geq1 = const.tile([128, 128], f32, tag="geq1")
        nc.gpsimd.affine_select(out=geq1[:], in_=ones128[:],
                                compare_op=is_ge, fill=0.0,
                                base=-1, channel_multiplier=-1,
                                pattern=[[1, 128]])
        id_f = const.tile([128, 128], f32, tag="id_f")
        nc.vector.tensor_sub(id_f[:], geq0[:], geq1[:])
        negid = const.tile([128, 128], bf16, tag="negid")
        nc.vector.tensor_scalar_mul(negid[:], id_f[:], -800.0)
        ltri = const.tile([128, 128], bf16, tag="ltri")
        nc.vector.tensor_scalar(out=ltri[:], in0=geq0[:],
                                scalar1=-1.0, scalar2=1.0,
                                op0=mult, op1=mybir.AluOpType.add)

        kTc = [const.tile([128, mc_n, 512], bf16, tag=f"kT{g}",
                          name=f"kT{g}") for g in range(sk // 512)]
        qTc = [const.tile([128, mc_n, QCH], bf16, tag=f"qT{qc}",
                          name=f"qT{qc}") for qc in range(qc_n)]
        vc = [const.tile([128, hpc, vw], bf16, tag=f"v{t}",
                         name=f"v{t}") for t in range(kt_n)]
        cxc = [const.tile([128, mc_n, QCH], bf16, tag=f"cx{qc}",
                          name=f"cx{qc}") for qc in range(qc_n)]

        # ---------------- V projection unit (one 128-token tile; natural
        # layout, padding mask folded in; everything off the scalar engine)
        def vproj_t(t):
            pv = prjp.tile([128, dkc], f32, tag="pj", name="pv")
            for c in range(kc_n):
                xt, c0 = xv[c][t // 4]
                o = c0 + (t % 4) * 128
                nc.tensor.matmul(pv[:], xt[:, o:o + 128],
                                 wv_sb[:, c, :],
                                 start=(c == 0), stop=(c == kc_n - 1))
            nc.vector.tensor_scalar(
                out=vc[t][:, :, 0:DK],
                in0=pv[:].rearrange("p (h k) -> p h k", h=hpc),
                scalar1=mask01[:, t:t + 1], scalar2=None, op0=mult)
            nc.vector.tensor_copy(
                vc[t][:, :, DK:vw],
                mask01p[:, t:t + 1].unsqueeze(1).broadcast_to([128, hpc, 1]))

        # ---------------- K projection unit (one 512-token chunk, one
        # feature block; packed [feature, tok] layout, plain-copy eviction)
        def kproj_u(qc, m):
            pk = prjp.tile([128, 512], f32, tag="pj", name="pk")
            for c in range(kc_n):
                xt, c0 = xk[c][qc]
                nc.tensor.matmul(
                    pk[:], wk_sb[:, c, m * 128:(m + 1) * 128],
                    xt[:, c0:c0 + 512],
                    start=(c == 0), stop=(c == kc_n - 1))
            nc.vector.tensor_copy(kTc[qc][:, m, :], pk[:])

        # ---------------- Q projection for one 512-chunk, one block
        def qproj_u(qc, m):
            pk = prjp.tile([128, 512], f32, tag="pj", name="pk")
            for c in range(kc_n):
                xt, c0 = xq[c][qc]
                nc.tensor.matmul(
                    pk[:], wq_sb[:, c, m * 128:(m + 1) * 128],
                    xt[:, c0:c0 + 512],
                    start=(c == 0), stop=(c == kc_n - 1))
            nc.vector.tensor_copy(qTc[qc][:, m, :], pk[:])

        # ---------------- attention for one 512-chunk (both head pairs in
        # one unit stream so there is no pair-boundary pipeline bubble).
        # `fillers` holds projection/output-projection unit callbacks
        # drained between kt units to fill the exp-paced PE slack.  The
        # post-softmax normalize is split: the PSUM-freeing quick-evict is
        # emitted as soon as a pair's accumulation ends; the reciprocal/
        # broadcast/scale tail is returned as callbacks for the NEXT
        # chunk's filler stream (cxc is only needed by oproj a chunk later).
        def attn_chunk(qc, fillers):
            q0 = qc * QCH
            nkt = (q0 + QCH) // 128
            units = [(m, kt) for m in range(mc_n) for kt in range(nkt)]
            rate = -(-len(fillers) // len(units))
            ctxs = {}
            deferred = []
            tails = []

            def mk_norm_tail(cbs, m):
                bcs = {}

                def prep():
                    for hh in (0, 1):
                        dn = nrm.tile([1, QCH], f32, tag="dn", name="dn")
                        nc.vector.tensor_copy(dn[:], cbs[hh][DK:DK + 1, :])
                        rc = nrm.tile([1, QCH], f32, tag="rc", name="rc")
                        nc.vector.reciprocal_approx_fast(rc[:], dn[:])
                        bc = nrm.tile([64, QCH], f32, tag="bc", name="bc")
                        nc.gpsimd.partition_broadcast(bc[:], rc[:])
                        bcs[hh] = bc

                def tt(cols=slice(0, QCH)):
                    if not bcs:
                        prep()
                    for hh in (0, 1):
                        nc.vector.tensor_tensor(
                            out=cxc[qc][hh * 64:(hh + 1) * 64, m, cols],
                            in0=cbs[hh][0:DK, cols], in1=bcs[hh][:, cols],
                            op=mult)
                return tt

            def mk_av(pB, m, kt, off):
                def go():
                    for hh in (0, 1):
                        nc.tensor.matmul(
                            ctxs[m][hh][:, off:QCH],
                            vc[kt][:, 2 * m + hh, :],
                            pB[:, hh, off:QCH],
                            start=(kt == 0), stop=(kt == nkt - 1),
                            skip_group_check=True)
                    if kt == nkt - 1:
                        # pair m done: free the ctx PSUM banks now
                        cbs = []
                        for hh in (0, 1):
                            cb = cbp.tile([vw, QCH], f32, tag="cb",
                                          name="cb")
                            nc.vector.tensor_copy(cb[:], ctxs[m][hh][:])
                            cbs.append(cb)
                        tails.append(mk_norm_tail(cbs, m))
                return go

            for m, kt in units:
                if kt == 0:
                    ctxs[m] = [ctp.tile([vw, QCH], f32, tag="c", name="cx")
                               for _ in (0, 1)]
                wp = min(QCH, q0 + QCH - kt * 128)   # valid q width
                off = QCH - wp
                diag = kt >= nkt - 4
                sB = sbp.tile([128, 2, QCH], f32, tag="s", name="sB")
                for hh in (0, 1):
                    nc.tensor.matmul(
                        sB[:, hh, off:QCH],
                        kTc[kt // 4][hh * 64:(hh + 1) * 64, m,
                                     (kt % 4) * 128:(kt % 4 + 1) * 128],
                        qTc[qc][hh * 64:(hh + 1) * 64, m, off:QCH],
                        start=True, stop=not diag,
                        skip_group_check=True)
                    if diag:
                        nc.tensor.matmul(
                            sB[:, hh, off:off + 128], negid[:], ltri[:],
                            start=False, stop=True, skip_group_check=True)
                pB = ptp.tile([128, 2, QCH], bf16, tag="p", name="pB")
                nc.scalar.activation(pB[:, :, off:QCH], sB[:, :, off:QCH],
                                     Exp, scale=0.125)
                deferred.append(mk_av(pB, m, kt, off))
                for _ in range(rate):
                    if fillers:
                        fillers.pop(0)()
                    elif tails:
                        tails.pop(0)()
                while len(deferred) > 2:
                    deferred.pop(0)()
            for fn in deferred:
                fn()
            return tails

        # ---------------- output projection for a 128-token group.
        # Per-fc DMAs spread across queues; the final groups split further
        # so the last transfer does not dominate the kernel tail.
        def oproj_qt(qc, qt, fine=False):
            qg = qc * QCH + qt * 128
            o_sb = outp.tile([128, fc_n, 512], bf16, tag="o", name="o_sb")
            for fc in range(fc_n):
                po = prjp.tile([128, 512], f32, tag="pj", name="po")
                for m in range(mc_n):
                    nc.tensor.matmul(
                        po[:], cxc[qc][:, m, qt * 128:(qt + 1) * 128],
                        wo_sb[:, m, fc, :],
                        start=(m == 0), stop=(m == mc_n - 1))
                nc.vector.tensor_copy(o_sb[:, fc, :], po[:])
                cols = slice(fc * 512, (fc + 1) * 512)
                if fine:
                    # tail: split across rows and issue from the (by now
                    # idle) scalar queue to shorten the final drain
                    for rh in (0, 1):
                        rows = slice(rh * 64, (rh + 1) * 64)
                        nc.scalar.dma_start(
                            out_d[qg + rh * 64:qg + (rh + 1) * 64, cols],
                            o_sb[rows, fc, :])
                else:
                    nc.sync.dma_start(out_d[qg:qg + 128, cols],
                                      o_sb[:, fc, :])

        # ---------------- main schedule: K0/Q0 up-front so the exp stream
        # starts as early as possible; chunk 0's V tiles and every later
        # chunk's projections, the previous chunk's output projection, and
        # the previous chunk's normalize tails all drain as fillers inside
        # the attention unit streams.
        def proj_units(qc):
            us = [(lambda t=t: vproj_t(t))
                  for t in range(4 * qc, min(4 * qc + 4, kt_n))]
            us += [(lambda m=m: kproj_u(qc, m)) for m in range(mc_n)]
            us += [(lambda m=m: qproj_u(qc, m)) for m in range(mc_n)]
            return us

        for t in range(min(4, kt_n)):
            vproj_t(t)
        for m in range(mc_n):
            kproj_u(0, m)
        for m in range(mc_n):
            qproj_u(0, m)
        tails = []
        for qc in range(qc_n):
            fillers = []
            fillers += [(lambda f=f: f()) for f in tails]
            if qc > 0:
                fillers += [(lambda qt=qt, q=qc - 1: oproj_qt(q, qt))
                            for qt in range(QCH // 128)]
            if qc + 1 < qc_n:
                fillers += proj_units(qc + 1)
            tails = attn_chunk(qc, fillers)
            for u in fillers:
                u()
        # final chunk: interleave the per-qt slices of the remaining
        # normalize tails with the output projection so the tail shortens
        for qt in range(QCH // 128):
            cols = slice(qt * 128, (qt + 1) * 128)
            for f in tails:
                f(cols)
            oproj_qt(qc_n - 1, qt, fine=True)
    nc.compile()
    return nc


def _get_program(cfg):
    if cfg not in _PROG_CACHE:
        _PROG_CACHE[cfg] = _build(cfg)
    return _PROG_CACHE[cfg]


def _shard_inputs(query, key, value, mask, Wq, Wk, Wv, Wo):
    """Build the 8 per-core input maps."""
    import ml_dtypes
    f = ml_dtypes.bfloat16
    in_maps = []
    xt = {}
    for b in range(B):
        xt[b] = (np.ascontiguousarray(query[b].T).astype(f),
                 np.ascontiguousarray(key[b].T).astype(f),
                 np.ascontiguousarray(value[b].T).astype(f),
                 np.ascontiguousarray(mask[b], dtype=np.int32))
    for c in range(N_CORES):
        b, hg = divmod(c, CORES_PER_BATCH)
        rows = slice(hg * DKC, (hg + 1) * DKC)
        xq, xk, xv, mb = xt[b]
        in_maps.append({
            "xqT": xq, "xkT": xk, "xvT": xv, "maskb": mb,
            "wq": np.ascontiguousarray(Wq[rows, :].T).astype(f),
            "wk": np.ascontiguousarray(Wk[rows, :].T).astype(f),
            "wv": np.ascontiguousarray(Wv[rows, :].T).astype(f),
            "wo": np.ascontiguousarray(Wo[:, rows].T).astype(f),
        })
    return in_maps


def kernel(query, key, value, mask, Wq, Wk, Wv, Wo):
    from concourse.bass_utils import run_bass_kernel_spmd

    nc = _get_program((SQ, SK, D, DKC))
    in_maps = _shard_inputs(np.asarray(query), np.asarray(key),
                            np.asarray(value), np.asarray(mask),
                            np.asarray(Wq), np.asarray(Wk),
                            np.asarray(Wv), np.asarray(Wo))
    res = run_bass_kernel_spmd(nc, in_maps, list(range(N_CORES)))
    out = np.zeros((B, SQ, D), dtype=np.float32)
    for c in range(N_CORES):
        out[c // CORES_PER_BATCH] += res.results[c]["out"].astype(np.float32)
    return out


# revision 73
# speedup vs baseline: 1.0932x; 1.0932x over previous
"""Multi-head attention (B=2, SQ=SK=2048, D=1024, H=16, DK=64) on 8 TRN2 cores.

Sharding: core c handles batch b = c//4 and head-group hg = c%4 (4 heads,
256 feature columns of each projection).  Each core computes its heads'
Q/K/V projections, causal+padding-masked softmax attention, and a partial
output projection; the host sums the 4 partials per batch.

All matmul operands are bf16 (1 cycle/row on the PE).  Device layouts:
  qT/kT  [128, m, tok]  packed: feature block m holds heads 2m (partitions
                        0-63) and 2m+1 (64-127) -- exactly the projection
                        psum layout, so evictions are plain copies.
  v      [tok, dk+1]    natural per head, padding mask folded into the rows;
                        the extra "masked ones" column makes the AV matmul
                        emit the softmax denominator for free.
  sT     [ktok, qtok]   transposed scores in PSUM; the two heads of a pair
                        run as K=64 row-tiled matmuls (partitions 0-63 /
                        64-127) that execute concurrently in the PE array.
  ctxT   [65, qtok]     accumulated over ktok tiles (row 64 = denominator).

Causality is exploited at 128-token granularity: score/AV/exp work for a
k-tile only covers valid queries (free dim trimmed), and the diagonal
128x128 triangle is zeroed via affine_select after exp.  Softmax runs
without max subtraction (scores are O(6) for randn inputs).  The Q
projection is emitted per 512-token chunk, interleaved with attention, so
the scalar engine's exp stream starts early; the output projection of
chunk qc-1 is interleaved into chunk qc's attention to fill PE gaps.
"""

import numpy as np

B, SQ, SK, D, H, DK = 2, 2048, 2048, 1024, 16, 64
N_CORES = 8
CORES_PER_BATCH = 4
DKC = D // CORES_PER_BATCH          # 256 projection columns per core
QCH = 512                           # q-chunk (moving free dim)
ONES_EPS = 1e-20

_PROG_CACHE = {}


def _build(cfg):
    """Build the per-core Bass program. cfg = (sq, sk, d, dkc)."""
    import concourse.bass as bass  # noqa: F401
    import concourse.mybir as mybir
    import concourse.tile as tile
    from concourse import bacc
    from contextlib import ExitStack

    f32 = mybir.dt.float32
    bf16 = mybir.dt.bfloat16
    i32 = mybir.dt.int32
    Exp = mybir.ActivationFunctionType.Exp
    mult = mybir.AluOpType.mult
    is_ge = mybir.AluOpType.is_ge

    sq, sk, d, dkc = cfg
    kc_n = d // 128                  # contraction chunks for projections
    mc_n = dkc // 128                # head pairs (128-feature blocks)
    kt_n = sk // 128                 # key tiles
    qc_n = sq // QCH                 # q chunks
    hpc = dkc // DK                  # heads per core
    vw = DK + 1                      # v row width per head incl. ones col
    fc_n = d // 512                  # output feature chunks

    nc = bacc.Bacc("TRN2", target_bir_lowering=False, debug=False,
                   enable_asserts=False, num_devices=N_CORES)

    xqT = nc.dram_tensor("xqT", [d, sq], bf16, kind="ExternalInput").ap()
    xkT = nc.dram_tensor("xkT", [d, sk], bf16, kind="ExternalInput").ap()
    xvT = nc.dram_tensor("xvT", [d, sk], bf16, kind="ExternalInput").ap()
    wq_d = nc.dram_tensor("wq", [d, dkc], bf16, kind="ExternalInput").ap()
    wk_d = nc.dram_tensor("wk", [d, dkc], bf16, kind="ExternalInput").ap()
    wv_d = nc.dram_tensor("wv", [d, dkc], bf16, kind="ExternalInput").ap()
    wo_d = nc.dram_tensor("wo", [dkc, d], bf16, kind="ExternalInput").ap()
    mask_d = nc.dram_tensor("maskb", [sk], i32, kind="ExternalInput").ap()
    out_d = nc.dram_tensor("out", [sq, d], bf16, kind="ExternalOutput").ap()

    with tile.TileContext(nc) as tc, ExitStack() as ctx:
        const = ctx.enter_context(tc.tile_pool(name="const", bufs=1))
        wpool = ctx.enter_context(tc.tile_pool(name="wpool", bufs=4))
        xpool = ctx.enter_context(tc.tile_pool(name="xpool", bufs=1))
        ptp = ctx.enter_context(tc.tile_pool(name="ptp", bufs=4))
        outp = ctx.enter_context(tc.tile_pool(name="outp", bufs=2))
        nrm = ctx.enter_context(tc.tile_pool(name="nrm", bufs=2))
        cbp = ctx.enter_context(tc.tile_pool(name="cbp", bufs=2))
        sbp = ctx.enter_context(tc.tile_pool(name="sbp", bufs=2,
                                             space="PSUM"))
        prjp = ctx.enter_context(tc.tile_pool(name="prjp", bufs=2,
                                              space="PSUM"))
        ctp = ctx.enter_context(tc.tile_pool(name="ctp", bufs=2,
                                             space="PSUM"))

        # ---------------- DMA everything up-front.  xv is issued from the
        # (otherwise idle) vector engine at fine grain so the V projection
        # starts early; xk/xq go on sync after the weights.  Each dma_start
        # lands on one HW queue (~22 GB/s), so tensors are split into many
        # sub-DMAs that spread across the 16 queues.
        mask_i = const.tile([128, kt_n], i32, tag="mask_i")
        nc.sync.dma_start(mask_i[:], mask_d.rearrange("(t p) -> p t", p=128))

        wv_sb = wpool.tile([128, kc_n, dkc], bf16, tag="w")
        wv_r = wv_d.rearrange("(c p) m -> p c m", p=128)
        wk_sb = wpool.tile([128, kc_n, dkc], bf16, tag="w")
        wk_r = wk_d.rearrange("(c p) m -> p c m", p=128)
        wq_sb = wpool.tile([128, kc_n, dkc], bf16, tag="w")
        wq_r = wq_d.rearrange("(c p) m -> p c m", p=128)
        wo_sb = wpool.tile([128, mc_n, fc_n, 512], bf16, tag="w")
        wo_r = wo_d.rearrange("(c p) (f n) -> p c f n", p=128, n=512)

        def w_issues(dst, src_r, n):
            return [lambda: nc.sync.dma_start(dst[:], src_r[:])]

        _xt = [0]

        def alloc_x(ntok):
            """One tile (and one striped DMA) per 512-token chunk, with the
            contraction-row blocks as a middle dim.  Entries are (tile, c)
            at 512-token granularity."""
            nch = ntok // 512
            xs = [[None] * nch for _ in range(kc_n)]
            tiles = []
            for t in range(nch):
                _xt[0] += 1
                tl = xpool.tile([128, kc_n, 512], bf16, tag=f"x{_xt[0]}",
                                name="xc")
                tiles.append(tl)
                for c in range(kc_n):
                    xs[c][t] = (tl, c)
            return xs, tiles

        xv, xv_g = alloc_x(sk)
        xk, xk_g = alloc_x(sk)
        xq, xq_g = alloc_x(sq)

        def x_issue(tiles, dram, t):
            if t >= len(tiles):
                return []
            r = dram.rearrange("(c p) t -> p c t", p=128)
            return [lambda: nc.sync.dma_start(
                tiles[t][:], r[:, :, t * 512:(t + 1) * 512])]

        issues = (w_issues(wv_sb, wv_r, kc_n) + x_issue(xv_g, xvT, 0)
                  + w_issues(wk_sb, wk_r, kc_n) + x_issue(xk_g, xkT, 0)
                  + w_issues(wq_sb, wq_r, kc_n) + x_issue(xq_g, xqT, 0)
                  + x_issue(xv_g, xvT, 1) + x_issue(xk_g, xkT, 1)
                  + x_issue(xq_g, xqT, 1)
                  + x_issue(xv_g, xvT, 2) + x_issue(xk_g, xkT, 2)
                  + w_issues(wo_sb, wo_r, mc_n)
                  + x_issue(xv_g, xvT, 3) + x_issue(xq_g, xqT, 2)
                  + x_issue(xk_g, xkT, 3) + x_issue(xq_g, xqT, 3))
        for fn in issues:
            fn()

        # ---------------- constants / persistent tensors
        mask01 = const.tile([128, kt_n], f32, tag="mask01")
        nc.vector.tensor_copy(mask01[:], mask_i[:])
        mask01p = const.tile([128, kt_n], f32, tag="mask01p")
        nc.vector.tensor_scalar_add(mask01p[:], mask01[:], ONES_EPS)
        # causal-bias constants: negid = -800*I, ltri = strict lower
        # triangle (ones below the diagonal).  The diagonal score tiles get
        # `negid.T @ ltri` (= -800 where q_local < k_local) accumulated into
        # PSUM before exp, so exp underflows the future positions to zero
        # without any gpsimd work on the critical path.
        ones128 = const.tile([128, 128], f32, tag="ones128")
        nc.vector.memset(ones128[:], 1.0)
        geq0 = const.tile([128, 128], f32, tag="geq0")
        nc.gpsimd.affine_select(out=geq0[:], in_=ones128[:],
                                compare_op=is_ge, fill=0.0,
                                base=0, channel_multiplier=-1,
                                pattern=[[1, 128]])
        geq1 = const.tile([128, 128], f32, tag="geq1")
        nc.gpsimd.affine_select(out=geq1[:], in_=ones128[:],
                                compare_op=is_ge, fill=0.0,
                                base=-1, channel_multiplier=-1,
                                pattern=[[1, 128]])
        id_f = const.tile([128, 128], f32, tag="id_f")
        nc.vector.tensor_sub(id_f[:], geq0[:], geq1[:])
        negid = const.tile([128, 128], bf16, tag="negid")
        nc.vector.tensor_scalar_mul(negid[:], id_f[:], -800.0)
        ltri = const.tile([128, 128], bf16, tag="ltri")
        nc.vector.tensor_scalar(out=ltri[:], in0=geq0[:],
                                scalar1=-1.0, scalar2=1.0,
                                op0=mult, op1=mybir.AluOpType.add)

        kTc = [const.tile([128, mc_n, 512], bf16, tag=f"kT{g}",
                          name=f"kT{g}") for g in range(sk // 512)]
        qTc = [const.tile([128, mc_n, QCH], bf16, tag=f"qT{qc}",
                          name=f"qT{qc}") for qc in range(qc_n)]
        vc = [const.tile([128, hpc, vw], bf16, tag=f"v{t}",
                         name=f"v{t}") for t in range(kt_n)]
        cxc = [const.tile([128, mc_n, QCH], bf16, tag=f"cx{qc}",
                          name=f"cx{qc}") for qc in range(qc_n)]

        # ---------------- V projection unit (one 128-token tile; natural
        # layout, padding mask folded in; everything off the scalar engine)
        def vproj_t(t):
            pv = prjp.tile([128, dkc], f32, tag="pj", name="pv")
            for c in range(kc_n):
                xt, cc = xv[c][t // 4]
                o = (t % 4) * 128
                nc.tensor.matmul(pv[:], xt[:, cc, o:o + 128],
                                 wv_sb[:, c, :],
                                 start=(c == 0), stop=(c == kc_n - 1))
            nc.vector.tensor_scalar(
                out=vc[t][:, :, 0:DK],
                in0=pv[:].rearrange("p (h k) -> p h k", h=hpc),
                scalar1=mask01[:, t:t + 1], scalar2=None, op0=mult)
            nc.vector.tensor_copy(
                vc[t][:, :, DK:vw],
                mask01p[:, t:t + 1].unsqueeze(1).broadcast_to([128, hpc, 1]))

        # ---------------- K projection unit (one 512-token chunk, one
        # feature block; packed [feature, tok] layout, plain-copy eviction)
        def kproj_u(qc, m):
            pk = prjp.tile([128, 512], f32, tag="pj", name="pk")
            for c in range(kc_n):
                xt, cc = xk[c][qc]
                nc.tensor.matmul(
                    pk[:], wk_sb[:, c, m * 128:(m + 1) * 128],
                    xt[:, cc, :],
                    start=(c == 0), stop=(c == kc_n - 1))
            nc.vector.tensor_copy(kTc[qc][:, m, :], pk[:])

        # ---------------- Q projection for one 512-chunk, one block
        def qproj_u(qc, m):
            pk = prjp.tile([128, 512], f32, tag="pj", name="pk")
            for c in range(kc_n):
                xt, cc = xq[c][qc]
                nc.tensor.matmul(
                    pk[:], wq_sb[:, c, m * 128:(m + 1) * 128],
                    xt[:, cc, :],
                    start=(c == 0), stop=(c == kc_n - 1))
            nc.vector.tensor_copy(qTc[qc][:, m, :], pk[:])

        # ---------------- attention for one 512-chunk (both head pairs in
        # one unit stream so there is no pair-boundary pipeline bubble).
        # `fillers` holds projection/output-projection unit callbacks
        # drained between kt units to fill the exp-paced PE slack.  The
        # post-softmax normalize is split: the PSUM-freeing quick-evict is
        # emitted as soon as a pair's accumulation ends; the reciprocal/
        # broadcast/scale tail is returned as callbacks for the NEXT
        # chunk's filler stream (cxc is only needed by oproj a chunk later).
        def attn_chunk(qc, fillers):
            q0 = qc * QCH
            nkt = (q0 + QCH) // 128
            units = [(m, kt) for m in range(mc_n) for kt in range(nkt)]
            rate = -(-len(fillers) // len(units))
            ctxs = {}
            deferred = []
            tails = []

            def mk_norm_tail(cbs, m):
                bcs = {}

                def prep():
                    for hh in (0, 1):
                        dn = nrm.tile([1, QCH], f32, tag="dn", name="dn")
                        nc.vector.tensor_copy(dn[:], cbs[hh][DK:DK + 1, :])
                        rc = nrm.tile([1, QCH], f32, tag="rc", name="rc")
                        nc.vector.reciprocal_approx_fast(rc[:], dn[:])
                        bc = nrm.tile([64, QCH], f32, tag="bc", name="bc")
                        nc.gpsimd.partition_broadcast(bc[:], rc[:])
                        bcs[hh] = bc

                def tt(cols=slice(0, QCH)):
                    if not bcs:
                        prep()
                    for hh in (0, 1):
                        nc.vector.tensor_tensor(
                            out=cxc[qc][hh * 64:(hh + 1) * 64, m, cols],
                            in0=cbs[hh][0:DK, cols], in1=bcs[hh][:, cols],
                            op=mult)
                return tt

            def mk_av(pB, m, kt, off):
                def go():
                    for hh in (0, 1):
                        nc.tensor.matmul(
                            ctxs[m][hh][:, off:QCH],
                            vc[kt][:, 2 * m + hh, :],
                            pB[:, hh, off:QCH],
                            start=(kt == 0), stop=(kt == nkt - 1),
                            skip_group_check=True)
                    if kt == nkt - 1:
                        # pair m done: free the ctx PSUM banks now
                        cbs = []
                        for hh in (0, 1):
                            cb = cbp.tile([vw, QCH], f32, tag="cb",
                                          name="cb")
                            nc.vector.tensor_copy(cb[:], ctxs[m][hh][:])
                            cbs.append(cb)
                        tails.append(mk_norm_tail(cbs, m))
                return go

            for m, kt in units:
                if kt == 0:
                    ctxs[m] = [ctp.tile([vw, QCH], f32, tag="c", name="cx")
                               for _ in (0, 1)]
                wp = min(QCH, q0 + QCH - kt * 128)   # valid q width
                off = QCH - wp
                diag = kt >= nkt - 4
                sB = sbp.tile([128, 2, QCH], f32, tag="s", name="sB")
                for hh in (0, 1):
                    nc.tensor.matmul(
                        sB[:, hh, off:QCH],
                        kTc[kt // 4][hh * 64:(hh + 1) * 64, m,
                                     (kt % 4) * 128:(kt % 4 + 1) * 128],
                        qTc[qc][hh * 64:(hh + 1) * 64, m, off:QCH],
                        start=True, stop=not diag,
                        skip_group_check=True)
                    if diag:
                        nc.tensor.matmul(
                            sB[:, hh, off:off + 128], negid[:], ltri[:],
                            start=False, stop=True, skip_group_check=True)
                pB = ptp.tile([128, 2, QCH], bf16, tag="p", name="pB")
                nc.scalar.activation(pB[:, :, off:QCH], sB[:, :, off:QCH],
                                     Exp, scale=0.125)
                deferred.append(mk_av(pB, m, kt, off))
                for _ in range(rate):
                    if fillers:
                        fillers.pop(0)()
                    elif tails:
                        tails.pop(0)()
                while len(deferred) > 2:
                    deferred.pop(0)()
            for fn in deferred:
                fn()
            return tails

        # ---------------- output projection for a 128-token group.
        # Per-fc DMAs spread across queues; the final groups split further
        # so the last transfer does not dominate the kernel tail.
        def oproj_qt(qc, qt, fine=False):
            qg = qc * QCH + qt * 128
            o_sb = outp.tile([128, fc_n, 512], bf16, tag="o", name="o_sb")
            for fc in range(fc_n):
                po = prjp.tile([128, 512], f32, tag="pj", name="po")
                for m in range(mc_n):
                    nc.tensor.matmul(
                        po[:], cxc[qc][:, m, qt * 128:(qt + 1) * 128],
                        wo_sb[:, m, fc, :],
                        start=(m == 0), stop=(m == mc_n - 1))
                nc.vector.tensor_copy(o_sb[:, fc, :], po[:])
                cols = slice(fc * 512, (fc + 1) * 512)
                if fine:
                    # tail: split across rows and issue from the (by now
                    # idle) scalar queue to shorten the final drain
                    for rh in (0, 1):
                        rows = slice(rh * 64, (rh + 1) * 64)
                        nc.scalar.dma_start(
                            out_d[qg + rh * 64:qg + (rh + 1) * 64, cols],
                            o_sb[rows, fc, :])
                else:
                    nc.sync.dma_start(out_d[qg:qg + 128, cols],
                                      o_sb[:, fc, :])

        # ---------------- main schedule: K0/Q0 up-front so the exp stream
        # starts as early as possible; chunk 0's V tiles and every later
        # chunk's projections, the previous chunk's output projection, and
        # the previous chunk's normalize tails all drain as fillers inside
        # the attention unit streams.
        def proj_units(qc):
            us = [(lambda t=t: vproj_t(t))
                  for t in range(4 * qc, min(4 * qc + 4, kt_n))]
            us += [(lambda m=m: kproj_u(qc, m)) for m in range(mc_n)]
            us += [(lambda m=m: qproj_u(qc, m)) for m in range(mc_n)]
            return us

        for t in range(min(4, kt_n)):
            vproj_t(t)
        for m in range(mc_n):
            kproj_u(0, m)
        for m in range(mc_n):
            qproj_u(0, m)
        tails = []
        for qc in range(qc_n):
            fillers = []
            fillers += [(lambda f=f: f()) for f in tails]
            if qc > 0:
                fillers += [(lambda qt=qt, q=qc - 1: oproj_qt(q, qt))
                            for qt in range(QCH // 128)]
            if qc + 1 < qc_n:
                fillers += proj_units(qc + 1)
            tails = attn_chunk(qc, fillers)
            for u in fillers:
                u()
        # final chunk: interleave the per-qt slices of the remaining
        # normalize tails with the output projection so the tail shortens
        for qt in range(QCH // 128):
            cols = slice(qt * 128, (qt + 1) * 128)
            for f in tails:
                f(cols)
            oproj_qt(qc_n - 1, qt, fine=True)
    nc.compile()
    return nc


def _get_program(cfg):
    if cfg not in _PROG_CACHE:
        _PROG_CACHE[cfg] = _build(cfg)
    return _PROG_CACHE[cfg]


def _shard_inputs(query, key, value, mask, Wq, Wk, Wv, Wo):
    """Build the 8 per-core input maps."""
    import ml_dtypes
    f = ml_dtypes.bfloat16
    in_maps = []
    xt = {}
    for b in range(B):
        xt[b] = (np.ascontiguousarray(query[b].T).astype(f),
                 np.ascontiguousarray(key[b].T).astype(f),
                 np.ascontiguousarray(value[b].T).astype(f),
                 np.ascontiguousarray(mask[b], dtype=np.int32))
    for c in range(N_CORES):
        b, hg = divmod(c, CORES_PER_BATCH)
        rows = slice(hg * DKC, (hg + 1) * DKC)
        xq, xk, xv, mb = xt[b]
        in_maps.append({
            "xqT": xq, "xkT": xk, "xvT": xv, "maskb": mb,
            "wq": np.ascontiguousarray(Wq[rows, :].T).astype(f),
            "wk": np.ascontiguousarray(Wk[rows, :].T).astype(f),
            "wv": np.ascontiguousarray(Wv[rows, :].T).astype(f),
            "wo": np.ascontiguousarray(Wo[:, rows].T).astype(f),
        })
    return in_maps


def kernel(query, key, value, mask, Wq, Wk, Wv, Wo):
    from concourse.bass_utils import run_bass_kernel_spmd

    nc = _get_program((SQ, SK, D, DKC))
    in_maps = _shard_inputs(np.asarray(query), np.asarray(key),
                            np.asarray(value), np.asarray(mask),
                            np.asarray(Wq), np.asarray(Wk),
                            np.asarray(Wv), np.asarray(Wo))
    res = run_bass_kernel_spmd(nc, in_maps, list(range(N_CORES)))
    out = np.zeros((B, SQ, D), dtype=np.float32)
    for c in range(N_CORES):
        out[c // CORES_PER_BATCH] += res.results[c]["out"].astype(np.float32)
    return out
